# revision 2
# baseline (speedup 1.0000x reference)
"""Trainium2 Bass kernel for nn_DynamicsBase: multi-type one-hot scatter.

Computes out[f, a, 16*t + actions[f, t, a]] = 1.0 over a zero base of shape
[2048, 256, 128] f32. Frames are sharded across 8 NeuronCores (pure data
parallelism, no communication).

Per core the kernel is a raw-Bass (no TileContext) program, which skips
Tile's ~1 us prologue barrier and shrinks the epilogue:
  - actions arrive a-major as [128, 2, 256, 8] uint8 (partition = f%128,
    h = f//128) and load in three pieces so the first compare can start as
    soon as the first 16 a-columns land (+900 ns DMA sem prop).
  - DVE tensor_tensor is_equal against a j-iota constant using broadcast
    (step-0) access patterns produces each one-hot tile in SBUF.
  - SP-queue HWDGE stores stream tiles to HBM. The first four tiles cover
    4 a-columns each (728 ns stores — the smallest size that still
    pipelines gaplessly against the 650 ns per-DMA SEQ cost), the rest 8
    (1456 ns); the store stream is gapless from the first store on.
Manual semaphores: act_sem (DMA +16/load), cmp_sem (+1/compare), st_sem
(+16/store; also the o-buffer WAR reuse gate).

Cost-model timeline: 99.3 us vs 93.2 us HBM-store floor (32 MiB/core at
360 GB/s); the remaining gap is head latency (first-DMA chain + sem prop +
first compare + store issue ~4.9 us) and the 1.2 us completion tail.

Self-contained: hardcodes shapes; takes full inputs, returns full output.
"""
import numpy as np
from contextlib import ExitStack

import concourse.bacc as bacc
import concourse.bass as bass
import concourse.mybir as mybir

NUM_FRAMES, NUM_TYPES, NUM_ACTIONS = 2048, 8, 256
J = 16                      # sub-actions per type
TOTAL = NUM_TYPES * J       # 128 one-hot width
N_CORES = 8
F_PER_CORE = NUM_FRAMES // N_CORES  # 256

RAMP = (4, 4, 4, 4)         # AB sizes of the leading h=0 tiles
A0_COLS = 16                # a-columns in the first (latency-critical) load
BUFS = 8                    # o-tile ring depth

_CACHE = {}


def _build_nc():
    nc = bacc.Bacc("TRN2")
    act = nc.dram_tensor("actions_t", [128, 2, NUM_ACTIONS, NUM_TYPES],
                         mybir.dt.uint8, kind="ExternalInput")
    out = nc.dram_tensor("out", [F_PER_CORE, NUM_ACTIONS, TOTAL],
                         mybir.dt.float32, kind="ExternalOutput")

    tiles = []
    a = 0
    for ab in RAMP:
        tiles.append((0, a, ab))
        a += ab
    while a < NUM_ACTIONS:
        tiles.append((0, a, 8))
        a += 8
    for a in range(0, NUM_ACTIONS, 8):
        tiles.append((1, a, 8))
    n_tiles = len(tiles)

    with ExitStack() as ctx:
        block = ctx.enter_context(nc.Block("main"))
        act_sb = ctx.enter_context(
            nc.sbuf_tensor("act_sb", [128, 2 * NUM_ACTIONS * NUM_TYPES],
                           mybir.dt.uint8))
        cmod_sb = ctx.enter_context(
            nc.sbuf_tensor("cmod_sb", [128, J], mybir.dt.uint8))
        obufs = [ctx.enter_context(
            nc.sbuf_tensor(f"o{i}", [128, 8 * TOTAL], mybir.dt.float32))
            for i in range(BUFS)]
        act_sem = ctx.enter_context(nc.semaphore("act_sem"))
        cmp_sem = ctx.enter_context(nc.semaphore("cmp_sem"))
        st_sem = ctx.enter_context(nc.semaphore("st_sem"))

        act_v = act_sb[:, :].rearrange("p (h a t) -> p h a t", h=2,
                                       a=NUM_ACTIONS)

        # act tier per tile: 1 = first A0_COLS of h0; 2 = rest of h0; 3 = h1
        def tier(k):
            h, a, ab = tiles[k]
            if h == 1:
                return 3
            return 1 if a + ab <= A0_COLS else 2

        @block.sync
        def _(sp):
            sp.dma_start(act_v[:, 0, 0:A0_COLS],
                         act[:, 0, 0:A0_COLS]).then_inc(act_sem, 16)
            sp.dma_start(act_v[:, 0, A0_COLS:],
                         act[:, 0, A0_COLS:]).then_inc(act_sem, 16)
            sp.dma_start(act_v[:, 1], act[:, 1]).then_inc(act_sem, 16)
            for k, (h, a, ab) in enumerate(tiles):
                sp.wait_ge(cmp_sem, k + 1)
                o = obufs[k % BUFS]
                dst = out[h * 128:(h + 1) * 128, a:a + ab, :]
                src = o[:, 0:ab * TOTAL].rearrange("p (a c) -> p a c",
                                                   c=TOTAL)
                sp.dma_start(dst, src).then_inc(st_sem, 16)
            sp.wait_ge(st_sem, 16 * n_tiles)

        @block.vector
        def _(dve):
            for j in range(J):
                dve.memset(cmod_sb[:, j:j + 1], j)
            cur_tier = 0
            for k, (h, a, ab) in enumerate(tiles):
                t = tier(k)
                if t > cur_tier:
                    dve.wait_ge(act_sem, 16 * t)
                    cur_tier = t
                if k >= BUFS:
                    dve.wait_ge(st_sem, 16 * (k - BUFS + 1))
                o = obufs[k % BUFS]
                in1 = (act_v[:, h, a:a + ab, :]
                       .unsqueeze(3).broadcast_to([128, ab, NUM_TYPES, J]))
                in0 = (cmod_sb[:, :].unsqueeze(1).unsqueeze(1)
                       .broadcast_to([128, ab, NUM_TYPES, J]))
                o_ap = o[:, 0:ab * TOTAL].rearrange(
                    "p (a t j) -> p a t j", t=NUM_TYPES, j=J)
                dve.tensor_tensor(o_ap, in0, in1,
                                  op=mybir.AluOpType.is_equal
                                  ).then_inc(cmp_sem, 1)

    nc.compile()
    return nc


def _get_nc():
    if "nc" not in _CACHE:
        _CACHE["nc"] = _build_nc()
    return _CACHE["nc"]


def _get_runner():
    """Build (once) a cached PJRT executor for the SPMD bass program.

    Mirrors concourse.bass_utils.run_bass_kernel_spmd's axon path
    (bass2jax.run_bass_via_pjrt) but caches the jitted shard_map callable so
    repeated kernel() calls don't re-trace/re-compile (~10 s each)."""
    if "runner" in _CACHE:
        return _CACHE["runner"]

    import jax
    from jax.sharding import Mesh, PartitionSpec
    from jax.experimental.shard_map import shard_map
    from concourse import bass2jax

    nc = _get_nc()
    bass2jax.install_neuronx_cc_hook()

    partition_name = (nc.partition_id_tensor.name
                      if nc.partition_id_tensor else None)
    in_names, out_names, out_avals, zero_shapes = [], [], [], []
    for alloc in nc.m.functions[0].allocations:
        if not isinstance(alloc, mybir.MemoryLocationSet):
            continue
        name = alloc.memorylocations[0].name
        if alloc.kind == "ExternalInput":
            if name != partition_name:
                in_names.append(name)
        elif alloc.kind == "ExternalOutput":
            shape = tuple(alloc.tensor_shape)
            dtype = mybir.dt.np(alloc.dtype)
            out_names.append(name)
            out_avals.append(jax.core.ShapedArray(shape, dtype))
            zero_shapes.append((shape, dtype))
    n_params = len(in_names)
    all_in_names = list(in_names) + list(out_names)
    if partition_name is not None:
        all_in_names.append(partition_name)
    donate = tuple(range(n_params, n_params + len(out_names)))

    def _body(*args):
        operands = list(args)
        if partition_name is not None:
            operands.append(bass2jax.partition_id_tensor())
        outs = bass2jax._bass_exec_p.bind(
            *operands,
            out_avals=tuple(out_avals),
            in_names=tuple(all_in_names),
            out_names=tuple(out_names),
            lowering_input_output_aliases=(),
            sim_require_finite=True,
            sim_require_nnan=True,
            nc=nc,
        )
        return tuple(outs)

    devices = jax.devices()[:N_CORES]
    mesh = Mesh(np.asarray(devices), ("core",))
    n_io = n_params + len(out_names)
    sharded = jax.jit(
        shard_map(_body, mesh=mesh,
                  in_specs=(PartitionSpec("core"),) * n_io,
                  out_specs=(PartitionSpec("core"),) * len(out_names),
                  check_rep=False),
        donate_argnums=donate, keep_unused=True)

    runner = {
        "sharded": sharded,
        "in_names": in_names,
        "out_names": out_names,
        "zero_shapes": zero_shapes,
    }
    _CACHE["runner"] = runner
    return runner


def _shard_actions(actions):
    """actions [2048, 8, 256] int -> [1024, 2, 256, 8] uint8: per core the
    frames split into partition = f%128 / h = f//128 and the (t, a) axes
    transpose to a-major so any a-column slice is contiguous. Values are
    0..15 so uint8 is exact and shrinks the load 4x vs int32."""
    a = actions.astype(np.uint8).reshape(N_CORES, 2, 128, NUM_TYPES,
                                         NUM_ACTIONS)
    return np.ascontiguousarray(
        a.transpose(0, 2, 1, 4, 3).reshape(N_CORES * 128, 2, NUM_ACTIONS,
                                           NUM_TYPES))


def _run_fallback(act_global):
    """Stock path via run_bass_kernel_spmd (re-jits per call, so only used
    if the cached PJRT runner path fails)."""
    from concourse.bass_utils import run_bass_kernel_spmd
    nc = _get_nc()
    in_maps = [{"actions_t": act_global[128 * c:128 * (c + 1)]}
               for c in range(N_CORES)]
    res = run_bass_kernel_spmd(nc, in_maps, core_ids=list(range(N_CORES)))
    return np.concatenate([r["out"] for r in res.results], axis=0)


def kernel(actions, base):
    actions = np.asarray(actions)
    base = np.asarray(base)
    assert actions.shape == (NUM_FRAMES, NUM_TYPES, NUM_ACTIONS), actions.shape
    act_global = _shard_actions(actions)
    out = None
    try:
        r = _get_runner()
        assert r["in_names"] == ["actions_t"] and r["out_names"] == ["out"]
        (shape, dtype), = r["zero_shapes"]
        zeros = np.zeros((N_CORES * shape[0], *shape[1:]), dtype)
        out_global, = r["sharded"](act_global, zeros)
        out = np.asarray(out_global)
        if np.isnan(out).any():  # transient axon flake seen once; retry
            out = None
    except Exception:
        out = None
    if out is None:
        out = _run_fallback(act_global)
    out = out.reshape(NUM_FRAMES, NUM_ACTIONS, TOTAL)
    return out.astype(base.dtype, copy=False)
